# revision 15
# baseline (speedup 1.0000x reference)
"""Trainium2 Bass kernel for GQA causal attention (B=2, S=2048, D=2048,
16 q-heads / 4 kv-heads, head_dim=128, interleaved RoPE).

Sharding: DP=2 over batch x TP=4 over head groups (8 cores).
Core c: batch b=c//4, rank r=c%4 -> q-heads [4r,4r+4), kv-head r.
Each core computes its heads' attention output (transposed layout [e,s]),
an AllToAll within each 4-core group reshards heads->sequence, and each
core then runs the full output projection for its S/4 sequence rows.
Host-side work is layout only: slicing, transposing, bf16 casting.
"""

import math
import sys

sys.path.insert(0, "/opt/trn_rl_repo")

import ml_dtypes
import numpy as np

import concourse.bass as bass
import concourse.mybir as mybir
import concourse.tile as tile
from concourse import bacc
from concourse.bass_utils import run_bass_kernel_spmd
from concourse.masks import make_identity

BF16 = mybir.dt.bfloat16
F32 = mybir.dt.float32

N_HEADS = 16
N_KV_HEADS = 4
HD = 128
ROPE_THETA = 10000.0
TP = 4
N_CORES = 8


def build_graph(S=2048, D=2048, HQL=4, NS=512):
    """Build the per-core SPMD graph. HQL = local q heads; kv local = 1.

    Layouts (per core):
      xT   [D, S]  bf16   x[b].T
      wqT  [D, HQL*HD] bf16  (rows of wq pre-permuted even/odd per head)
      wkT  [D, HD] bf16      (pre-permuted)
      wvT  [D, HD] bf16
      woT  [TP*HQL*HD, D] bf16  (full wo.T, head-major rows)
      cc/ss [64, S] bf16     rope cos/sin tables (row i = freq i)
      mask [128, NS+384] bf16 causal staircase master
      out  [S//TP, D] f32
    """
    hd = HD
    ND = D // 128          # d-tiles (contraction tiles for projections)
    NC = S // NS           # s-chunks
    NK = S // 128          # sk-tiles
    MQ = HQL * hd          # local q width
    DIAG = NS // 128       # sk-tiles per chunk that need a causal mask
    NB = N_CORES // TP     # batches (= DP groups)
    OW = S // N_CORES      # out columns owned per core per batch
    scale = 1.0 / math.sqrt(hd)

    nc = bacc.Bacc("TRN2", target_bir_lowering=False, debug=False,
                   num_devices=N_CORES)

    xT_e = nc.dram_tensor("xT", [D, S], BF16, kind="ExternalInput").ap()
    wqT_e = nc.dram_tensor("wqT", [D, MQ], BF16, kind="ExternalInput").ap()
    wkT_e = nc.dram_tensor("wkT", [D, hd], BF16, kind="ExternalInput").ap()
    wvT_e = nc.dram_tensor("wvT", [D, hd], BF16, kind="ExternalInput").ap()
    woT_e = nc.dram_tensor("woT", [TP * MQ, D], BF16, kind="ExternalInput").ap()
    cc_e = nc.dram_tensor("cc", [64, S], BF16, kind="ExternalInput").ap()
    ss_e = nc.dram_tensor("ss", [64, S], BF16, kind="ExternalInput").ap()
    mask_e = nc.dram_tensor("mask", [128, NS + 384], BF16,
                            kind="ExternalInput").ap()
    out_e = nc.dram_tensor("out", [NB * OW, D], F32,
                           kind="ExternalOutput").ap()

    # 8-way AllToAll: shard d = my heads x out-cols [d*OW,(d+1)*OW) of my
    # batch. Every core owns cols [c*OW,(c+1)*OW) of BOTH batches.
    a2a_in = nc.dram_tensor("a2a_in", [N_CORES * MQ, OW], BF16)
    a2a_out = nc.dram_tensor("a2a_out", [N_CORES * MQ, OW], BF16)
    groups = [list(range(N_CORES))]

    from contextlib import ExitStack

    with tile.TileContext(nc) as tc:
        with ExitStack() as ctx:
            ep = ctx.enter_context
            xt_pool = ep(tc.tile_pool(name="xt", bufs=ND))
            wq_pool = ep(tc.tile_pool(name="wq", bufs=ND))
            wkv_pool = ep(tc.tile_pool(name="wkv", bufs=2 * ND))
            const_pool = ep(tc.tile_pool(name="const", bufs=1))
            rt_pool = ep(tc.tile_pool(name="rt", bufs=HQL + 1))
            vst_pool = ep(tc.tile_pool(name="vst", bufs=1))
            tmp_pool = ep(tc.tile_pool(name="tmp", bufs=4))
            stage_pool = ep(tc.tile_pool(name="stg", bufs=2))
            odown_pool = ep(tc.tile_pool(name="odown", bufs=2))
            ropeo_pool = ep(tc.tile_pool(name="ropeo", bufs=2))
            pt_pool = ep(tc.tile_pool(name="pt", bufs=3))
            den_pool = ep(tc.tile_pool(name="den", bufs=1))
            denb_pool = ep(tc.tile_pool(name="denb", bufs=2))
            recip_pool = ep(tc.tile_pool(name="recip", bufs=2))
            rbc_pool = ep(tc.tile_pool(name="rbc", bufs=2))
            attn_pool = ep(tc.tile_pool(name="attn", bufs=4))
            ao_pool = ep(tc.tile_pool(name="ao", bufs=2 * TP * HQL))
            wo_pool = ep(tc.tile_pool(name="wo", bufs=2))
            osb_pool = ep(tc.tile_pool(name="osb", bufs=2))
            ps_pool = ep(tc.tile_pool(name="ps", bufs=8, space="PSUM"))
            # ---- constants ----
            ident = const_pool.tile([128, 128], BF16, tag="ident")
            make_identity(nc, ident[:])
            ones = const_pool.tile([128, 1], BF16, tag="ones")
            nc.gpsimd.memset(ones[:], 1.0)
            cc = const_pool.tile([64, S], BF16, tag="cc")
            ss = const_pool.tile([64, S], BF16, tag="ss")
            msk = const_pool.tile([128, NS + 384], BF16, tag="msk")
            nc.sync.dma_start(cc[:], cc_e[:])
            nc.sync.dma_start(ss[:], ss_e[:])
            nc.sync.dma_start(msk[:], mask_e[:])

            # ---- load xT + weights ----
            xts = []
            for d in range(ND):
                t = xt_pool.tile([128, S], BF16, tag="xt")
                nc.sync.dma_start(t[:], xT_e[d * 128:(d + 1) * 128, :])
                xts.append(t)
            wqs = []
            for d in range(ND):
                t = wq_pool.tile([128, MQ], BF16, tag="wq")
                nc.sync.dma_start(t[:], wqT_e[d * 128:(d + 1) * 128, :])
                wqs.append(t)
            wks, wvs = [], []
            for d in range(ND):
                t = wkv_pool.tile([128, hd], BF16, tag="wkv")
                nc.sync.dma_start(t[:], wkT_e[d * 128:(d + 1) * 128, :])
                wks.append(t)
            for d in range(ND):
                t = wkv_pool.tile([128, hd], BF16, tag="wkv")
                nc.sync.dma_start(t[:], wvT_e[d * 128:(d + 1) * 128, :])
                wvs.append(t)

            # ---- projections + rope ----
            # m-tiles: HQL q heads, then k, then v. Each m-tile is 128 wide.
            rts = []     # stacked roped q tiles [128, S] bf16 (+k last)
            vst = vst_pool.tile([128, S], BF16, tag="vst")  # vT staging

            def proj_mtile(lhs_tiles, mslice, is_v, rt_tile):
                for s in range(NC):
                    ps = ps_pool.tile([128, NS], F32, tag="ps")
                    for d in range(ND):
                        nc.tensor.matmul(
                            ps[:], lhs_tiles[d][:, mslice],
                            xts[d][:, s * NS:(s + 1) * NS],
                            start=(d == 0), stop=(d == ND - 1))
                    if is_v:
                        nc.scalar.copy(vst[:, s * NS:(s + 1) * NS], ps[:])
                    else:
                        # rope: rows 0:64 = even comps (t0), 64:128 = odd (t1)
                        # DMA cannot read PSUM -> stage through SBUF on ACT.
                        ssl = slice(s * NS, (s + 1) * NS)
                        stg = stage_pool.tile([128, NS], F32, tag="stg")
                        nc.scalar.copy(stg[:], ps[:])
                        od = odown_pool.tile([64, NS], F32, tag="odown")
                        nc.sync.dma_start(od[:], stg[64:128, :])
                        t0c = tmp_pool.tile([64, NS], F32, tag="tmp")
                        t1s = tmp_pool.tile([64, NS], F32, tag="tmp")
                        nc.vector.tensor_mul(t0c[:], stg[0:64, :], cc[:, ssl])
                        nc.vector.tensor_mul(t1s[:], od[:], ss[:, ssl])
                        nc.vector.tensor_sub(rt_tile[0:64, ssl], t0c[:], t1s[:])
                        t0s = tmp_pool.tile([64, NS], F32, tag="tmp")
                        t1c = tmp_pool.tile([64, NS], F32, tag="tmp")
                        nc.vector.tensor_mul(t0s[:], stg[0:64, :], ss[:, ssl])
                        nc.vector.tensor_mul(t1c[:], od[:], cc[:, ssl])
                        ro = ropeo_pool.tile([64, NS], BF16, tag="ropeo")
                        nc.vector.tensor_add(ro[:], t0s[:], t1c[:])
                        nc.sync.dma_start(rt_tile[64:128, ssl], ro[:])

            for h in range(HQL):
                rt = rt_pool.tile([128, S], BF16, tag="rt")
                proj_mtile(wqs, slice(h * hd, (h + 1) * hd), False, rt)
                rts.append(rt)
            krt = rt_pool.tile([128, S], BF16, tag="rt")
            proj_mtile(wks, slice(0, hd), False, krt)
            proj_mtile(wvs, slice(0, hd), True, None)

            # ---- v transpose: vst [e, s] -> vnat [sk, e] blocks ----
            vnat = vst_pool.tile([128, S], BF16, tag="vnat")
            for st in range(NK):
                tp = ps_pool.tile([128, 128], BF16, tag="ps")
                nc.tensor.transpose(
                    tp[:], vst[:, st * 128:(st + 1) * 128], ident[:])
                nc.scalar.copy(vnat[:, st * 128:(st + 1) * 128], tp[:])

            # ---- attention (transposed flash, no max subtraction) ----
            for h in range(HQL):
                qrt = rts[h]
                at_ps = []
                for j in range(NC):
                    at_ps.append(ps_pool.tile([128, NS], F32, tag="ps",
                                              name=f"atps_h{h}_j{j}"))
                den = den_pool.tile([128, S], F32, tag="den")
                nc.vector.memset(den[:], 0.0)
                for si in range(NK):
                    jmin_valid = si // DIAG  # chunks j >= si*128//NS
                    for j in range(jmin_valid, NC):
                        sl = slice(j * NS, (j + 1) * NS)
                        sc = ps_pool.tile([128, NS], F32, tag="ps")
                        nc.tensor.matmul(
                            sc[:], krt[:, si * 128:(si + 1) * 128],
                            qrt[:, sl], start=True, stop=True)
                        pt = pt_pool.tile([128, NS], BF16, tag="pt")
                        nc.scalar.activation(
                            pt[:], sc[:], mybir.ActivationFunctionType.Exp,
                            scale=scale)
                        o = si * 128 - j * NS
                        if o >= 0:  # diagonal block: causal mask
                            nc.vector.tensor_mul(
                                pt[:], pt[:],
                                msk[:, (NS - 128) - o:(2 * NS - 128) - o])
                        nc.vector.tensor_add(den[:, sl], den[:, sl], pt[:])
                        nc.tensor.matmul(
                            at_ps[j][:], vnat[:, si * 128:(si + 1) * 128],
                            pt[:], start=(si == 0),
                            stop=(si == (j + 1) * DIAG - 1))
                # normalize + export to a2a_in
                for j in range(NC):
                    sl = slice(j * NS, (j + 1) * NS)
                    dbf = denb_pool.tile([128, NS], BF16, tag="denb")
                    nc.vector.tensor_copy(dbf[:], den[:, sl])
                    dps = ps_pool.tile([1, NS], F32, tag="ps")
                    nc.tensor.matmul(dps[:], ones[:], dbf[:],
                                     start=True, stop=True)
                    rc = recip_pool.tile([1, NS], F32, tag="recip")
                    nc.vector.reciprocal(rc[:], dps[:])
                    rbc = rbc_pool.tile([128, NS], F32, tag="rbc")
                    nc.gpsimd.partition_broadcast(rbc[:], rc[:])
                    asb = attn_pool.tile([128, NS], BF16, tag="attn")
                    nc.vector.tensor_mul(asb[:], at_ps[j][:], rbc[:])
                    # export chunk j (abs cols [j*NS,(j+1)*NS)) into per-shard
                    # slots of a2a_in (shard width OW may differ from NS)
                    a = j * NS
                    while a < (j + 1) * NS:
                        shard = a // OW
                        b = min((j + 1) * NS, (shard + 1) * OW)
                        nc.sync.dma_start(
                            a2a_in.ap()[shard * MQ + h * hd:
                                        shard * MQ + (h + 1) * hd,
                                        a % OW:a % OW + (b - a)],
                            asb[:, a - j * NS:b - j * NS])
                        a = b

            # ---- AllToAll within each group of TP cores ----
            nc.gpsimd.collective_compute(
                "AllToAll", mybir.AluOpType.bypass,
                ins=[a2a_in.ap().opt()], outs=[a2a_out.ap().opt()],
                replica_groups=groups)

            # ---- output projection: out[sq, :] = attnT_all.T @ woT ----
            NH = TP * HQL  # 16 global head tiles
            aos = {}
            for beta in range(NB):
                for ht in range(NH):
                    row0 = (beta * TP + ht // HQL) * MQ + (ht % HQL) * hd
                    t = ao_pool.tile([128, OW], BF16, tag="ao",
                                     name=f"ao_{beta}_{ht}")
                    nc.sync.dma_start(t[:], a2a_out.ap()[row0:row0 + 128, :])
                    aos[(beta, ht)] = t
            sq_tiles = [(beta, t) for beta in range(NB)
                        for t in range(OW // 128)]
            NO = D // NS             # dout chunks (4)
            TPAIR = 2
            for tpp in range(len(sq_tiles) // TPAIR):
                pair = sq_tiles[tpp * TPAIR:(tpp + 1) * TPAIR]
                wo_tiles = []
                for ht in range(NH):
                    w = wo_pool.tile([128, D], BF16, tag="wo")
                    nc.sync.dma_start(
                        w[:], woT_e[ht * 128:(ht + 1) * 128, :])
                    wo_tiles.append(w)
                pso = [[ps_pool.tile([128, NS], F32, tag="ps",
                                     name=f"pso_{tpp}_{t}_{n}")
                        for n in range(NO)] for t in range(TPAIR)]
                for ht in range(NH):
                    for ti, (beta, t) in enumerate(pair):
                        for n in range(NO):
                            nc.tensor.matmul(
                                pso[ti][n][:],
                                aos[(beta, ht)][:, t * 128:(t + 1) * 128],
                                wo_tiles[ht][:, n * NS:(n + 1) * NS],
                                start=(ht == 0), stop=(ht == NH - 1))
                for ti, (beta, t) in enumerate(pair):
                    r0 = beta * OW + t * 128
                    for n in range(NO):
                        ob = osb_pool.tile([128, NS], F32, tag="osb")
                        nc.scalar.copy(ob[:], pso[ti][n][:])
                        nc.sync.dma_start(
                            out_e[r0:r0 + 128, n * NS:(n + 1) * NS], ob[:])

    nc.compile()
    return nc


def host_prepare(x, wq, wk, wv, wo, S, D, HQL, NS):
    """Layout-only host prep: slice/transpose/cast + rope tables + mask."""
    hd = HD
    MQ = HQL * hd
    n_heads = wq.shape[0] // hd
    n_kv = wk.shape[0] // hd
    bf = ml_dtypes.bfloat16

    # per-head even/odd row permutation for q/k
    perm = np.concatenate([np.arange(0, hd, 2), np.arange(1, hd, 2)])

    def permute_heads(w):
        nh = w.shape[0] // hd
        w = w.reshape(nh, hd, -1)[:, perm, :]
        return w.reshape(nh * hd, -1)

    wq_p = permute_heads(wq)
    wk_p = permute_heads(wk)

    inv_freq = 1.0 / (ROPE_THETA ** (np.arange(0, hd, 2, dtype=np.float64) / hd))
    ang = np.arange(S, dtype=np.float64)[None, :] * inv_freq[:, None]  # [64,S]
    cc = np.cos(ang).astype(bf)
    ss = np.sin(ang).astype(bf)

    # causal staircase master: msk[p, c] = 1 if p <= c - (NS-128)
    p = np.arange(128)[:, None]
    c = np.arange(NS + 384)[None, :]
    msk = (p <= c - (NS - 128)).astype(bf)

    woT = np.ascontiguousarray(wo.T).astype(bf)

    B = x.shape[0]
    in_maps = []
    for core in range(N_CORES):
        b = core // TP
        r = core % TP
        qsl = slice(r * MQ, (r + 1) * MQ)
        ksl = slice(r * hd, (r + 1) * hd)
        in_maps.append({
            "xT": np.ascontiguousarray(x[b].T).astype(bf),
            "wqT": np.ascontiguousarray(wq_p[qsl].T).astype(bf),
            "wkT": np.ascontiguousarray(wk_p[ksl].T).astype(bf),
            "wvT": np.ascontiguousarray(wv[ksl].T).astype(bf),
            "woT": woT,
            "cc": cc, "ss": ss, "mask": msk,
        })
    return in_maps


_NC_CACHE = {}


def get_graph(S=2048, D=2048, HQL=4, NS=512):
    key = (S, D, HQL, NS)
    if key not in _NC_CACHE:
        _NC_CACHE[key] = build_graph(S, D, HQL, NS)
    return _NC_CACHE[key]


def kernel(x, wq, wk, wv, wo, trace=False):
    B, S, D = x.shape
    HQL = (wq.shape[0] // HD) // TP
    NS = 512
    nc = get_graph(S, D, HQL, NS)
    in_maps = host_prepare(x, wq, wk, wv, wo, S, D, HQL, NS)
    res = run_bass_kernel_spmd(nc, in_maps, core_ids=list(range(N_CORES)),
                               trace=trace)
    out = np.empty((B, S, D), dtype=np.float32)
    OW = S // N_CORES
    for core in range(N_CORES):
        r = res.results[core]["out"]
        for beta in range(B):
            out[beta, core * OW:(core + 1) * OW, :] = \
                r[beta * OW:(beta + 1) * OW, :]
    if trace:
        kernel.last_exec_time_ns = res.exec_time_ns
        kernel.last_results = res
    return out


# revision 26
# speedup vs baseline: 1.3597x; 1.3597x over previous
"""Trainium2 Bass kernel for GQA causal attention (B=2, S=2048, D=2048,
16 q-heads / 4 kv-heads, head_dim=128, interleaved RoPE).

Sharding: DP=2 over batch x TP=4 over head groups (8 cores).
Core c: batch b=c//4, rank r=c%4 -> q-heads [4r,4r+4), kv-head r.
Each core computes its heads' attention output (transposed layout [e,s]),
two column-strip AllToAlls reshard heads->sequence (overlapped with the
tail of attention), and each core runs the full output projection for its
512 strided sequence rows. Host-side work is layout only: slicing,
transposing, bf16 casting.
"""

import math
import sys

sys.path.insert(0, "/opt/trn_rl_repo")

from contextlib import ExitStack

import ml_dtypes
import numpy as np

import concourse.bass as bass
import concourse.mybir as mybir
import concourse.tile as tile
from concourse import bacc
from concourse.bass_utils import run_bass_kernel_spmd
from concourse.masks import make_identity

BF16 = mybir.dt.bfloat16
F32 = mybir.dt.float32

N_HEADS = 16
N_KV_HEADS = 4
HD = 128
ROPE_THETA = 10000.0
TP = 4
N_CORES = 8


def build_graph(S=2048, D=2048, HQL=4, NS=512):
    """Per-core SPMD graph. HQL = local q heads; local kv heads = 1.

    Output ownership is strided by 128-col strips: core c owns sequence
    cols {c*128 + m*1024} of both batches; strip set m is exchanged by
    AllToAll #m as soon as the first half of attention chunks finish.
    """
    hd = HD
    ND = D // 128          # d-tiles (projection contraction tiles)
    NC = S // NS           # s-chunks
    NK = S // 128          # sk-tiles
    MQ = HQL * hd          # local q width
    DIAG = NS // 128       # sk-tiles per chunk needing a causal mask
    NB = N_CORES // TP     # batches
    OW = S // N_CORES      # out cols per core per batch
    NM = max(1, S // (N_CORES * 128))   # strips (AllToAll count)
    SW = OW // NM          # strip width (=128 at full size)
    scale = 1.0 / math.sqrt(hd)
    NH = TP * HQL          # global head count

    nc = bacc.Bacc("TRN2", target_bir_lowering=False, debug=False,
                   num_devices=N_CORES)

    xT_e = nc.dram_tensor("xT", [D, S], BF16, kind="ExternalInput").ap()
    wqT_e = nc.dram_tensor("wqT", [D, MQ], BF16, kind="ExternalInput").ap()
    wkT_e = nc.dram_tensor("wkT", [D, hd], BF16, kind="ExternalInput").ap()
    wvT_e = nc.dram_tensor("wvT", [D, hd], BF16, kind="ExternalInput").ap()
    woT_e = nc.dram_tensor("woT", [NH * hd, D], BF16,
                           kind="ExternalInput").ap()
    cc_e = nc.dram_tensor("cc", [64, S], BF16, kind="ExternalInput").ap()
    ss_e = nc.dram_tensor("ss", [64, S], BF16, kind="ExternalInput").ap()
    mask_e = nc.dram_tensor("mask", [128, NS + 384], BF16,
                            kind="ExternalInput").ap()
    out_e = nc.dram_tensor("out", [NB * OW, D], F32,
                           kind="ExternalOutput").ap()

    a2a_in = [nc.dram_tensor(f"a2a_in{m}", [N_CORES * MQ, SW], BF16)
              for m in range(NM)]
    a2a_out = [nc.dram_tensor(f"a2a_out{m}", [N_CORES * MQ, SW], BF16)
               for m in range(NM)]
    groups = [list(range(N_CORES))]

    with tile.TileContext(nc) as tc, ExitStack() as ctx:
        ep = ctx.enter_context
        const_pool = ep(tc.tile_pool(name="const", bufs=1))
        rt_pool = ep(tc.tile_pool(name="rt", bufs=HQL + 1))
        vst_pool = ep(tc.tile_pool(name="vst", bufs=1))
        pt_pool = ep(tc.tile_pool(name="pt", bufs=9))
        recip_pool = ep(tc.tile_pool(name="recip", bufs=2))
        rbc_pool = ep(tc.tile_pool(name="rbc", bufs=2))
        attn_pool = ep(tc.tile_pool(name="attn", bufs=4))
        osb_pool = ep(tc.tile_pool(name="osb", bufs=3))
        ps_pool = ep(tc.tile_pool(name="ps", bufs=8, space="PSUM"))

        # ---- constants ----
        ident = const_pool.tile([128, 128], BF16, tag="ident")
        make_identity(nc, ident[:])
        ones = const_pool.tile([128, 32], BF16, tag="ones")
        nc.gpsimd.memset(ones[:], 1.0)
        cc = const_pool.tile([64, S], BF16, tag="cc")
        ss = const_pool.tile([64, S], BF16, tag="ss")
        msk = const_pool.tile([128, NS + 384], BF16, tag="msk")
        nc.sync.dma_start(cc[:], cc_e[:])
        nc.sync.dma_start(ss[:], ss_e[:])
        nc.sync.dma_start(msk[:], mask_e[:])

        rts = []
        vst = vst_pool.tile([128, S], BF16, tag="vst")   # vT staging
        vnat = vst_pool.tile([128, S], BF16, tag="vnat")  # v [sk, e] blocks

        # ---- phase 1: projections + rope (xt pools close after) ----
        with tc.tile_pool(name="xt", bufs=ND) as xt_pool, \
             tc.tile_pool(name="wq", bufs=ND) as wq_pool, \
             tc.tile_pool(name="wkv", bufs=2 * ND) as wkv_pool, \
             tc.tile_pool(name="tmp", bufs=4) as tmp_pool, \
             tc.tile_pool(name="stg", bufs=2) as stage_pool, \
             tc.tile_pool(name="odown", bufs=2) as odown_pool, \
             tc.tile_pool(name="ropeo", bufs=2) as ropeo_pool:
            xts, wqs, wks, wvs = [], [], [], []
            # interleaved loads so d-tile 0 arrives first -> PE starts early
            for d in range(ND):
                xt = xt_pool.tile([128, S], BF16, tag="xt",
                                  name=f"xt{d}")
                nc.sync.dma_start(xt[:], xT_e[d * 128:(d + 1) * 128, :])
                xts.append(xt)
                wq = wq_pool.tile([128, MQ], BF16, tag="wq", name=f"wq{d}")
                nc.sync.dma_start(wq[:], wqT_e[d * 128:(d + 1) * 128, :])
                wqs.append(wq)
                wk = wkv_pool.tile([128, hd], BF16, tag="wkv",
                                   name=f"wk{d}")
                nc.sync.dma_start(wk[:], wkT_e[d * 128:(d + 1) * 128, :])
                wks.append(wk)
                wv = wkv_pool.tile([128, hd], BF16, tag="wkv",
                                   name=f"wv{d}")
                nc.sync.dma_start(wv[:], wvT_e[d * 128:(d + 1) * 128, :])
                wvs.append(wv)

            def proj_mtile(lhs_tiles, mslice, is_v, rt_tile):
                for s in range(NC):
                    ps = ps_pool.tile([128, NS], F32, tag="ps", name="psp")
                    for d in range(ND):
                        nc.tensor.matmul(
                            ps[:], lhs_tiles[d][:, mslice],
                            xts[d][:, s * NS:(s + 1) * NS],
                            start=(d == 0), stop=(d == ND - 1))
                    if is_v:
                        nc.scalar.copy(vst[:, s * NS:(s + 1) * NS], ps[:])
                    else:
                        # rope; even comps in rows 0:64, odd in 64:128
                        ssl = slice(s * NS, (s + 1) * NS)
                        stg = stage_pool.tile([128, NS], F32, tag="stg")
                        nc.scalar.copy(stg[:], ps[:])
                        od = odown_pool.tile([64, NS], F32, tag="odown")
                        nc.sync.dma_start(od[:], stg[64:128, :])
                        t0c = tmp_pool.tile([64, NS], F32, tag="tmp")
                        t1s = tmp_pool.tile([64, NS], F32, tag="tmp")
                        nc.vector.tensor_mul(t0c[:], stg[0:64, :], cc[:, ssl])
                        nc.vector.tensor_mul(t1s[:], od[:], ss[:, ssl])
                        nc.vector.tensor_sub(rt_tile[0:64, ssl],
                                             t0c[:], t1s[:])
                        t0s = tmp_pool.tile([64, NS], F32, tag="tmp")
                        t1c = tmp_pool.tile([64, NS], F32, tag="tmp")
                        nc.vector.tensor_mul(t0s[:], stg[0:64, :], ss[:, ssl])
                        nc.vector.tensor_mul(t1c[:], od[:], cc[:, ssl])
                        ro = ropeo_pool.tile([64, NS], BF16, tag="ropeo")
                        nc.vector.tensor_add(ro[:], t0s[:], t1c[:])
                        nc.sync.dma_start(rt_tile[64:128, ssl], ro[:])

            for h in range(HQL):
                rt = rt_pool.tile([128, S], BF16, tag="rt", name=f"rtq{h}")
                proj_mtile(wqs, slice(h * hd, (h + 1) * hd), False, rt)
                rts.append(rt)
            krt = rt_pool.tile([128, S], BF16, tag="rt", name="rtk")
            proj_mtile(wks, slice(0, hd), False, krt)
            proj_mtile(wvs, slice(0, hd), True, None)

            # v transpose: vst [e, s] -> vnat [sk, e] blocks
            for st in range(NK):
                tpp = ps_pool.tile([128, 128], BF16, tag="ps", name="pst")
                nc.tensor.transpose(
                    tpp[:], vst[:, st * 128:(st + 1) * 128], ident[:])
                nc.scalar.copy(vnat[:, st * 128:(st + 1) * 128], tpp[:])

        # ---- woT preload (streams during attention; reuses xt space) ----
        wo_pool = ep(tc.tile_pool(name="wo", bufs=NH))
        ao_pool = ep(tc.tile_pool(name="ao", bufs=NB * NH * NM))
        wo_tiles = []
        for ht in range(NH):
            w = wo_pool.tile([128, D], BF16, tag="wo", name=f"wo{ht}")
            nc.sync.dma_start(w[:], woT_e[ht * 128:(ht + 1) * 128, :])
            wo_tiles.append(w)

        # ---- phase 2: attention (j outer so strips complete early) ----
        # heads processed in pairs so psum stays within 8 banks while each
        # head's softmax denominator gets its own [1, NS] bank (plain M=1
        # accumulation groups, no cross-group tricks)
        def attn_pass(j, heads):
            sl = slice(j * NS, (j + 1) * NS)
            at_ps = {h: ps_pool.tile([128, NS], F32, tag="ps",
                                     name=f"atps_j{j}_h{h}")
                     for h in heads}
            den_ps = {h: ps_pool.tile([1, NS], F32, tag="ps",
                                      name=f"den_j{j}_h{h}")
                      for h in heads}
            nsk = (j + 1) * DIAG

            def score_stage(si):
                o = si * 128 - j * NS
                pts = {}
                for h in heads:
                    sc = ps_pool.tile([128, NS], F32, tag="ps", name="psc")
                    nc.tensor.matmul(
                        sc[:], krt[:, si * 128:(si + 1) * 128],
                        rts[h][:, sl], start=True, stop=True)
                    pt = pt_pool.tile([128, NS], BF16, tag="pt")
                    nc.scalar.activation(
                        pt[:], sc[:], mybir.ActivationFunctionType.Exp,
                        scale=scale)
                    if o >= 0:  # diagonal block: causal mask
                        nc.vector.tensor_mul(
                            pt[:], pt[:],
                            msk[:, (NS - 128) - o:(2 * NS - 128) - o])
                    pts[h] = pt
                return pts

            def denav_stage(si, pts):
                for h in heads:
                    nc.tensor.matmul(
                        den_ps[h][:], ones[:, 0:1], pts[h][:],
                        start=(si == 0), stop=(si == nsk - 1))
                for h in heads:
                    nc.tensor.matmul(
                        at_ps[h][:], vnat[:, si * 128:(si + 1) * 128],
                        pts[h][:], start=(si == 0), stop=(si == nsk - 1))

            # 1-stage software pipeline: scores of si overlap den/av of si-1
            # so the PE never waits on the exp/mask of the current tile
            prev = None
            for si in range(nsk):
                pts = score_stage(si)
                if prev is not None:
                    denav_stage(*prev)
                prev = (si, pts)
            denav_stage(*prev)
            # normalize + export strip pieces
            for h in heads:
                rc = recip_pool.tile([1, NS], F32, tag="recip")
                nc.vector.reciprocal(rc[:], den_ps[h][:])
                rbc = rbc_pool.tile([128, NS], F32, tag="rbc")
                nc.gpsimd.partition_broadcast(rbc[:], rc[:])
                asb = attn_pool.tile([128, NS], BF16, tag="attn")
                nc.vector.tensor_mul(asb[:], at_ps[h][:], rbc[:])
                for i in range(NS // SW):
                    c = j * NS + i * SW
                    dd = (c // SW) % N_CORES
                    m = c // (N_CORES * SW)
                    nc.sync.dma_start(
                        a2a_in[m].ap()[dd * MQ + h * hd:
                                       dd * MQ + (h + 1) * hd, :],
                        asb[:, i * SW:(i + 1) * SW])

        def attn_chunk(j):
            for hp in range(0, HQL, 2):
                attn_pass(j, list(range(hp, min(hp + 2, HQL))))

        def do_a2a(m):
            nc.gpsimd.collective_compute(
                "AllToAll", mybir.AluOpType.bypass,
                ins=[a2a_in[m].ap().opt()], outs=[a2a_out[m].ap().opt()],
                replica_groups=groups)

        # chunks per strip: strip m complete after chunk (m+1)*NC/NM - 1
        per = NC // NM
        for m in range(NM):
            for j in range(m * per, (m + 1) * per):
                attn_chunk(j)
            do_a2a(m)

        # ---- phase 3: output projection ----
        NO = D // NS
        for m in range(NM):
            for beta in range(NB):
                aos = []
                for ht in range(NH):
                    row0 = (beta * TP + ht // HQL) * MQ + (ht % HQL) * hd
                    t = ao_pool.tile([128, SW], BF16, tag="ao",
                                     name=f"ao_{m}_{beta}_{ht}")
                    nc.sync.dma_start(
                        t[:], a2a_out[m].ap()[row0:row0 + 128, :])
                    aos.append(t)
                pso = [ps_pool.tile([128, NS], F32, tag="ps",
                                    name=f"pso_{m}_{beta}_{n}")
                       for n in range(NO)]
                for ht in range(NH):
                    for n in range(NO):
                        nc.tensor.matmul(
                            pso[n][:], aos[ht][:],
                            wo_tiles[ht][:, n * NS:(n + 1) * NS],
                            start=(ht == 0), stop=(ht == NH - 1))
                r0 = beta * OW + m * SW
                for n in range(NO):
                    ob = osb_pool.tile([128, NS], F32, tag="osb")
                    nc.scalar.copy(ob[:], pso[n][:])
                    nc.sync.dma_start(
                        out_e[r0:r0 + SW, n * NS:(n + 1) * NS], ob[0:SW, :])

    nc.compile()
    return nc


def host_prepare(x, wq, wk, wv, wo, S, D, HQL, NS):
    """Layout-only host prep: slice/transpose/cast + rope tables + mask."""
    hd = HD
    MQ = HQL * hd
    bf = ml_dtypes.bfloat16

    perm = np.concatenate([np.arange(0, hd, 2), np.arange(1, hd, 2)])

    def permute_heads(w):
        nh = w.shape[0] // hd
        w = w.reshape(nh, hd, -1)[:, perm, :]
        return w.reshape(nh * hd, -1)

    wq_p = permute_heads(wq)
    wk_p = permute_heads(wk)

    inv_freq = 1.0 / (ROPE_THETA ** (np.arange(0, hd, 2, dtype=np.float64)
                                     / hd))
    ang = np.arange(S, dtype=np.float64)[None, :] * inv_freq[:, None]
    cci = np.cos(ang).astype(bf)
    ssi = np.sin(ang).astype(bf)

    p = np.arange(128)[:, None]
    c = np.arange(NS + 384)[None, :]
    mski = (p <= c - (NS - 128)).astype(bf)

    woT = np.ascontiguousarray(wo.T).astype(bf)

    in_maps = []
    for core in range(N_CORES):
        b = core // TP
        r = core % TP
        qsl = slice(r * MQ, (r + 1) * MQ)
        ksl = slice(r * hd, (r + 1) * hd)
        in_maps.append({
            "xT": np.ascontiguousarray(x[b].T).astype(bf),
            "wqT": np.ascontiguousarray(wq_p[qsl].T).astype(bf),
            "wkT": np.ascontiguousarray(wk_p[ksl].T).astype(bf),
            "wvT": np.ascontiguousarray(wv[ksl].T).astype(bf),
            "woT": woT,
            "cc": cci, "ss": ssi, "mask": mski,
        })
    return in_maps


_NC_CACHE = {}


def get_graph(S=2048, D=2048, HQL=4, NS=512):
    key = (S, D, HQL, NS)
    if key not in _NC_CACHE:
        _NC_CACHE[key] = build_graph(S, D, HQL, NS)
    return _NC_CACHE[key]


def unshard_out(results, B, S, D):
    """results[core]["out"] is [NB*OW, D] with rows (beta, strip m, 128)."""
    out = np.empty((B, S, D), dtype=np.float32)
    OW = S // N_CORES
    NM = max(1, S // (N_CORES * 128))
    SW = OW // NM
    for core in range(N_CORES):
        r = results[core]["out"]
        for beta in range(B):
            for m in range(NM):
                c0 = core * SW + m * N_CORES * SW
                out[beta, c0:c0 + SW, :] = \
                    r[beta * OW + m * SW:beta * OW + (m + 1) * SW, :]
    return out


def kernel(x, wq, wk, wv, wo, trace=False):
    B, S, D = x.shape
    HQL = (wq.shape[0] // HD) // TP
    NS = 512
    nc = get_graph(S, D, HQL, NS)
    in_maps = host_prepare(x, wq, wk, wv, wo, S, D, HQL, NS)
    res = run_bass_kernel_spmd(nc, in_maps, core_ids=list(range(N_CORES)),
                               trace=trace)
    out = unshard_out(res.results, B, S, D)
    if trace:
        kernel.last_exec_time_ns = res.exec_time_ns
        kernel.last_results = res
    return out
